# revision 7
# baseline (speedup 1.0000x reference)
"""Causal temporal attention (B=4, T=2048, D=1024, H=16, hd=64) on 8 trn2 cores.

Sharding: core c handles batch b=c//2 and head-group hg=c%2 (8 heads, 512 dims).
Each core computes y_partial[b] = attn_out_g @ Wo_g.T for its head group; the
host sums the two partials per batch and adds bo.

Per-core dataflow:
  xT [1024, 2048] (host-pretransposed x[b]) streams in 256-col sub-chunks.
  qT,kT are computed transposed [512, T] (dims on partitions) so the S matmul
  contracts head_dim on partitions; v is computed natural [T, 512] with an
  appended ones-column per head so the AV matmul also produces the softmax
  denominator (row 64 of the [65, 512] accumulator).
  RMS-norm over head_dim (= partitions) uses a block-ones matmul for the
  sum-of-squares, ln/exp on ACT for rsqrt, and a broadcast matmul (with the
  norm weight folded in) to spread it back over partitions.
  Causality: tiles above the diagonal are skipped; boundary 128x128 blocks
  are masked by a triangular 0/1 multiply on GPSIMD after the exp.
All matmul inputs are float32r (TF32-like rounding, fp32 accumulation).
"""

import numpy as np

import concourse.bass as bass
import concourse.tile as tile
from concourse import bacc, mybir
from concourse.bass_utils import run_bass_kernel_spmd

F32 = mybir.dt.float32
F32R = mybir.dt.float32r
EXP = mybir.ActivationFunctionType.Exp
LN = mybir.ActivationFunctionType.Ln

EPS = 1e-6


def build_module(T=2048, with_qkbias=False, with_vbias=False, n_cores=8):
    """Build the per-core Bass module. D=1024, 8 heads of 64 dims per core."""
    D = 1024
    HG = 8          # heads per core
    HD = 64         # head dim
    DG = HG * HD    # 512 group dims
    NKT = T // 128  # k/t tiles
    NCH = T // 512  # q chunks
    SUB = 256       # xT streaming sub-chunk width

    nc = bacc.Bacc("TRN2", target_bir_lowering=False, debug=False,
                   num_devices=n_cores)

    xT_d = nc.dram_tensor("xt", [D, T], F32R, kind="ExternalInput")
    wq_d = nc.dram_tensor("wq", [D, DG], F32R, kind="ExternalInput")
    wk_d = nc.dram_tensor("wk", [D, DG], F32R, kind="ExternalInput")
    wv_d = nc.dram_tensor("wv", [D, DG], F32R, kind="ExternalInput")
    wo_d = nc.dram_tensor("wo", [DG, D], F32R, kind="ExternalInput")
    tri_d = nc.dram_tensor("tri", [128, 128], F32R, kind="ExternalInput")
    blk_d = nc.dram_tensor("blk", [128, 2], F32R, kind="ExternalInput")
    bcq_d = nc.dram_tensor("bcq", [2, 128], F32R, kind="ExternalInput")
    bck_d = nc.dram_tensor("bck", [2, 128], F32R, kind="ExternalInput")
    vones_d = nc.dram_tensor("vones", [128, HG], F32R, kind="ExternalInput")
    if with_qkbias:
        bq_d = nc.dram_tensor("bq", [4, 128], F32, kind="ExternalInput")
        bk_d = nc.dram_tensor("bk", [4, 128], F32, kind="ExternalInput")
    if with_vbias:
        bv_d = nc.dram_tensor("bv", [1, DG], F32R, kind="ExternalInput")
        ones1_d = nc.dram_tensor("ones1", [1, 128], F32R, kind="ExternalInput")
    y_d = nc.dram_tensor("y", [T, D], F32, kind="ExternalOutput")

    with nc.allow_low_precision(reason="float32r matmul inputs"), \
         tile.TileContext(nc) as tc:
        with (
            tc.tile_pool(name="res", bufs=1) as res,
            tc.tile_pool(name="ktp", bufs=1) as ktp,
            tc.tile_pool(name="vtp", bufs=1) as vtp,
            tc.tile_pool(name="st2", bufs=2) as st2,
            tc.tile_pool(name="st3", bufs=3) as st3,
            tc.tile_pool(name="st5", bufs=5) as st5,
            tc.tile_pool(name="qtp", bufs=2) as qtp,
            tc.tile_pool(name="psbig", bufs=2, space="PSUM") as psbig,
            tc.tile_pool(name="psmid", bufs=2, space="PSUM") as psmid,
            tc.tile_pool(name="pso", bufs=2, space="PSUM") as pso,
        ):
            # ---- resident loads ----
            wq_sb = res.tile([128, 8, DG], F32R, tag="wq")
            wk_sb = res.tile([128, 8, DG], F32R, tag="wk")
            wv_sb = res.tile([128, 8, DG], F32R, tag="wv")
            wo_sb = res.tile([128, 4, D], F32R, tag="wo")
            nc.sync.dma_start(out=wq_sb[:], in_=wq_d.ap().rearrange("(a p) m -> p a m", p=128))
            nc.sync.dma_start(out=wk_sb[:], in_=wk_d.ap().rearrange("(a p) m -> p a m", p=128))
            nc.sync.dma_start(out=wv_sb[:], in_=wv_d.ap().rearrange("(a p) m -> p a m", p=128))
            nc.sync.dma_start(out=wo_sb[:], in_=wo_d.ap().rearrange("(a p) m -> p a m", p=128))
            tri_sb = res.tile([128, 128], F32R, tag="tri")
            nc.sync.dma_start(out=tri_sb[:], in_=tri_d.ap())
            blk_sb = res.tile([128, 2], F32R, tag="blk")
            nc.sync.dma_start(out=blk_sb[:], in_=blk_d.ap())
            bcq_sb = res.tile([2, 128], F32R, tag="bcq")
            nc.sync.dma_start(out=bcq_sb[:], in_=bcq_d.ap())
            bck_sb = res.tile([2, 128], F32R, tag="bck")
            nc.sync.dma_start(out=bck_sb[:], in_=bck_d.ap())
            vones_sb = res.tile([128, HG], F32R, tag="vones")
            nc.sync.dma_start(out=vones_sb[:], in_=vones_d.ap())
            eps_sb = res.tile([2, 1], F32, tag="eps")
            nc.vector.memset(eps_sb[:], EPS)
            bq_sb = bk_sb = bv_sb = ones1_sb = None
            if with_qkbias:
                bq_sb = res.tile([128, 4], F32, tag="bq")
                nc.sync.dma_start(out=bq_sb[:], in_=bq_d.ap().rearrange("m p -> p m"))
                bk_sb = res.tile([128, 4], F32, tag="bk")
                nc.sync.dma_start(out=bk_sb[:], in_=bk_d.ap().rearrange("m p -> p m"))
            if with_vbias:
                bv_sb = res.tile([1, DG], F32R, tag="bv")
                nc.sync.dma_start(out=bv_sb[:], in_=bv_d.ap())
                ones1_sb = res.tile([1, 128], F32R, tag="ones1")
                nc.sync.dma_start(out=ones1_sb[:], in_=ones1_d.ap())

            # resident kT [dims, T] (4 tiles) and v [t, dims+ones] (NKT tiles)
            kt_sb = [ktp.tile([128, T], F32R, tag=f"kt{m}", name=f"kt{m}") for m in range(4)]
            v_sb = [vtp.tile([128, HG, HD + 1], F32R, tag=f"v{t}", name=f"v{t}")
                    for t in range(NKT)]

            xT_ap = xT_d.ap().rearrange("(a p) t -> p a t", p=128)

            for c in range(NCH):
                # ---------- projection phase for chunk c ----------
                xts = []
                for s in range(2):
                    xt = st3.tile([128, 8, SUB], F32R, tag="xt", bufs=2)
                    c0 = c * 512 + s * SUB
                    nc.sync.dma_start(out=xt[:], in_=xT_ap[:, :, c0:c0 + SUB])
                    xts.append(xt)

                qt_c = [qtp.tile([128, 512], F32R, tag=f"qt{m}", name=f"qt{m}", bufs=1)
                        for m in range(4)]

                # projection pipeline stages, skewed so PE never waits on
                # the DVE/ACT legs of the rms-norm chain.
                praw, psq = {}, {}

                def proj_qk(u):
                    which, m = u
                    w_sb = wq_sb if which == "q" else wk_sb
                    b_sb = bq_sb if which == "q" else bk_sb
                    ps = psmid.tile([128, 512], F32, tag="mid")
                    for s in range(2):
                        for k in range(8):
                            nc.tensor.matmul(
                                ps[:, s * SUB:(s + 1) * SUB],
                                w_sb[:, k, m * 128:(m + 1) * 128],
                                xts[s][:, k, :],
                                start=(k == 0), stop=(k == 7),
                            )
                    raw = st5.tile([128, 512], F32, tag="praw", bufs=4)
                    if b_sb is not None:
                        nc.vector.tensor_scalar_add(raw[:], ps[:],
                                                    b_sb[:, m:m + 1])
                    else:
                        nc.vector.tensor_copy(raw[:], ps[:])
                    sq = st3.tile([128, 512], F32R, tag="sq")
                    nc.vector.tensor_mul(sq[:], raw[:], raw[:])
                    praw[u] = raw
                    psq[u] = sq

                rs_t = {}

                def sumsq(u):
                    ssq = psmid.tile([2, 512], F32, tag="mid")
                    nc.tensor.matmul(ssq[:], blk_sb[:], psq[u][:],
                                     start=True, stop=True)
                    lssq = st2.tile([2, 512], F32, tag="lssq")
                    nc.scalar.activation(out=lssq[:], in_=ssq[:], func=LN,
                                         bias=eps_sb[:], scale=1.0 / HD)
                    rs = st2.tile([2, 512], F32R, tag="rs")
                    nc.scalar.activation(out=rs[:], in_=lssq[:], func=EXP,
                                         scale=-0.5)
                    rs_t[u] = rs

                def bcast_mul(u):
                    which, m = u
                    rsb = psbig.tile([128, 512], F32, tag="big")
                    nc.tensor.matmul(rsb[:],
                                     bcq_sb[:] if which == "q" else bck_sb[:],
                                     rs_t[u][:], start=True, stop=True)
                    if which == "q":
                        nc.vector.tensor_mul(qt_c[m][:], praw[u][:], rsb[:])
                    else:
                        nc.vector.tensor_mul(
                            kt_sb[m][:, c * 512:(c + 1) * 512],
                            praw[u][:], rsb[:])

                def proj_v(tt):
                    s, toff = divmod(tt * 128, SUB)
                    ps = psmid.tile([128, 512], F32, tag="mid")
                    for k in range(8):
                        nc.tensor.matmul(
                            ps[:],
                            xts[s][:, k, toff:toff + 128],
                            wv_sb[:, k, :],
                            start=(k == 0), stop=(k == 7 and not with_vbias),
                        )
                    if with_vbias:
                        nc.tensor.matmul(ps[:], ones1_sb[:], bv_sb[:],
                                         start=False, stop=True)
                    vt = v_sb[c * 4 + tt]
                    nc.vector.tensor_copy(
                        vt[:, :, 0:HD],
                        ps[:].rearrange("p (h d) -> p h d", h=HG),
                    )
                    nc.sync.dma_start(
                        out=vt[:, :, HD:HD + 1],
                        in_=vones_sb[:].rearrange("p (h o) -> p h o", o=1),
                    )

                units = [("q", m) for m in range(4)] + [("k", m) for m in range(4)]
                lead = [("p", u) for u in units] + [("v", t) for t in range(4)]
                # skewed emission: proj(u_i) | sumsq(u_{i-1}) | bcast(u_{i-3})
                for i in range(len(lead) + 3):
                    if i < len(lead):
                        kind, a = lead[i]
                        proj_qk(a) if kind == "p" else proj_v(a)
                    if 1 <= i <= len(units):
                        sumsq(units[i - 1])
                    if 3 <= i - 1 <= len(units) + 2 and 0 <= i - 4 < len(units):
                        bcast_mul(units[i - 4])

                # ---------- attention phase for chunk c ----------
                ot_c = [qtp.tile([128, 512], F32R, tag=f"ot{m}", name=f"ot{m}", bufs=1)
                        for m in range(4)]
                for h in range(HG):
                    mt = h // 2
                    r0 = (h % 2) * 64
                    o_ps = pso.tile([HD + 1, 512], F32, tag="o")
                    n_full = 4 * c

                    def s_mm(sp, col0, kt, q0, width):
                        # S^T tile: [k 0:128, q q0:q0+width], K = head_dim 64
                        nc.tensor.matmul(
                            sp[:, col0:col0 + width],
                            kt_sb[mt][r0:r0 + 64, kt * 128:(kt + 1) * 128],
                            qt_c[mt][r0:r0 + 64, q0:q0 + width],
                            start=True, stop=True,
                        )

                    def av_mm(es, col0, kt, q0, width, first, last):
                        nc.tensor.matmul(
                            o_ps[:, q0:q0 + width],
                            v_sb[kt][:, h, :],
                            es[:, col0:col0 + width],
                            start=first, stop=last,
                        )

                    # S tiles: pairs of full k-tiles, then 4 diagonal tiles.
                    # AV of group g-1 is emitted after S/exp of group g so the
                    # PE stream stays ahead of ACT.
                    pend = None
                    pend_es = None
                    for p0 in range(0, n_full, 2):
                        sp = psbig.tile([128, 1024], F32, tag="big")
                        s_mm(sp, 0, p0, 0, 512)
                        s_mm(sp, 512, p0 + 1, 0, 512)
                        es = st3.tile([128, 1024], F32R, tag="es", bufs=2)
                        nc.scalar.activation(out=es[:], in_=sp[:], func=EXP,
                                             scale=0.125)
                        if pend is not None:
                            for (col0, kt_, q0_, w_, fi, la) in pend:
                                av_mm(pend_es, col0, kt_, q0_, w_, fi, la)
                        pend_es = es
                        pend = [(0, p0, 0, 512, p0 == 0, False),
                                (512, p0 + 1, 0, 512, False, False)]
                    for jj in range(4):
                        kt = n_full + jj
                        q0 = 128 * jj
                        width = 512 - q0
                        sp = psbig.tile([128, 1024], F32, tag="big")
                        s_mm(sp, 0, kt, q0, width)
                        es = st3.tile([128, 1024], F32R, tag="es", bufs=2)
                        nc.scalar.activation(out=es[:, 0:width],
                                             in_=sp[:, 0:width],
                                             func=EXP, scale=0.125)
                        nc.gpsimd.tensor_mul(es[:, 0:128], es[:, 0:128],
                                             tri_sb[:])
                        if pend is not None:
                            for (col0, kt_, q0_, w_, fi, la) in pend:
                                av_mm(pend_es, col0, kt_, q0_, w_, fi, la)
                        pend_es = es
                        pend = [(0, kt, q0, width,
                                 jj == 0 and n_full == 0, jj == 3)]
                    for (col0, kt_, q0_, w_, fi, la) in pend:
                        av_mm(pend_es, col0, kt_, q0_, w_, fi, la)

                    # softmax normalization + transposed attention output
                    recip = st2.tile([1, 512], F32, tag="recip")
                    nc.vector.reciprocal(out=recip[:], in_=o_ps[64:65, :])
                    recipb = st2.tile([64, 512], F32, tag="recipb")
                    nc.gpsimd.partition_broadcast(recipb[:], recip[:])
                    nc.vector.tensor_mul(ot_c[mt][r0:r0 + 64, :],
                                         o_ps[0:64, :], recipb[:])

                # ---------- out-projection for chunk c ----------
                for tt in range(4):
                    for od in range(2):
                        yp = psmid.tile([128, 512], F32, tag="mid")
                        for m in range(4):
                            nc.tensor.matmul(
                                yp[:],
                                ot_c[m][:, tt * 128:(tt + 1) * 128],
                                wo_sb[:, m, od * 512:(od + 1) * 512],
                                start=(m == 0), stop=(m == 3),
                            )
                        ysb = st2.tile([128, 512], F32, tag="y")
                        nc.vector.tensor_copy(ysb[:], yp[:])
                        t0 = c * 512 + tt * 128
                        nc.sync.dma_start(
                            out=y_d.ap()[t0:t0 + 128, od * 512:(od + 1) * 512],
                            in_=ysb[:])

    nc.compile()
    return nc


_CACHE = {}


def _get_module(T, with_qkbias, with_vbias, n_cores):
    key = (T, with_qkbias, with_vbias, n_cores)
    if key not in _CACHE:
        _CACHE[key] = build_module(T, with_qkbias, with_vbias, n_cores)
    return _CACHE[key]


def make_consts(qn_w, kn_w):
    HG = 8
    tri = np.triu(np.ones((128, 128), np.float32))   # keep k<=q: [i <= j]
    blk = np.zeros((128, 2), np.float32)
    blk[0:64, 0] = 1.0
    blk[64:128, 1] = 1.0
    bcq = np.zeros((2, 128), np.float32)
    bck = np.zeros((2, 128), np.float32)
    bcq[0, 0:64] = qn_w
    bcq[1, 64:128] = qn_w
    bck[0, 0:64] = kn_w
    bck[1, 64:128] = kn_w
    vones = np.ones((128, HG), np.float32)
    return tri, blk, bcq, bck, vones


def make_in_maps(x, Wq, bq, Wk, bk, Wv, bv, Wo, qn_w, kn_w,
                 with_qkbias, with_vbias, n_cores=8):
    DG = 512
    tri, blk, bcq, bck, vones = make_consts(qn_w.astype(np.float32),
                                            kn_w.astype(np.float32))
    in_maps = []
    for c in range(n_cores):
        b, hg = divmod(c, 2)
        sl = slice(hg * DG, (hg + 1) * DG)
        im = {
            "xt": np.ascontiguousarray(x[b].T.astype(np.float32)),
            "wq": np.ascontiguousarray(Wq[sl, :].T.astype(np.float32)),
            "wk": np.ascontiguousarray(Wk[sl, :].T.astype(np.float32)),
            "wv": np.ascontiguousarray(Wv[sl, :].T.astype(np.float32)),
            "wo": np.ascontiguousarray(Wo[:, sl].T.astype(np.float32)),
            "tri": tri, "blk": blk, "bcq": bcq, "bck": bck, "vones": vones,
        }
        if with_qkbias:
            im["bq"] = bq[sl].astype(np.float32).reshape(4, 128)
            im["bk"] = bk[sl].astype(np.float32).reshape(4, 128)
        if with_vbias:
            im["bv"] = bv[sl].astype(np.float32).reshape(1, DG)
            im["ones1"] = np.ones((1, 128), np.float32)
        in_maps.append(im)
    return in_maps


def kernel(x, Wq, bq, Wk, bk, Wv, bv, Wo, bo, qn_w, kn_w):
    x = np.asarray(x); Wq = np.asarray(Wq); Wk = np.asarray(Wk)
    Wv = np.asarray(Wv); Wo = np.asarray(Wo)
    bq = np.asarray(bq); bk = np.asarray(bk); bv = np.asarray(bv)
    bo = np.asarray(bo)
    qn_w = np.asarray(qn_w); kn_w = np.asarray(kn_w)
    B, T, D = x.shape

    with_qkbias = bool(np.any(bq != 0) or np.any(bk != 0))
    with_vbias = bool(np.any(bv != 0))
    nc = _get_module(T, with_qkbias, with_vbias, 8)
    in_maps = make_in_maps(x, Wq, bq, Wk, bk, Wv, bv, Wo, qn_w, kn_w,
                           with_qkbias, with_vbias, 8)
    res = run_bass_kernel_spmd(nc, in_maps, core_ids=list(range(8)))
    out = np.empty((B, T, D), np.float32)
    for b in range(B):
        out[b] = res.results[2 * b]["y"] + res.results[2 * b + 1]["y"]
    out += bo.astype(np.float32)
    return out


# revision 22
# speedup vs baseline: 1.3047x; 1.3047x over previous
"""Causal temporal attention (B=4, T=2048, D=1024, H=16, hd=64) on 8 trn2 cores.

Sharding: core c handles batch b=c//2 and head-group hg=c%2 (8 heads, 512 dims).
Each core computes y_partial[b] = attn_out_g @ Wo_g.T for its head group; the
host sums the two partials per batch and adds bo.

Per-core dataflow:
  xT [1024, 2048] (host-pretransposed x[b]) streams in 256-col sub-chunks.
  qT,kT are computed transposed [512, T] (dims on partitions) so the S matmul
  contracts head_dim on partitions; v is computed natural [T, 512] with an
  appended ones-column per head so the AV matmul also produces the softmax
  denominator (row 64 of the [65, 512] accumulator).
  RMS-norm over head_dim (= partitions) uses a block-ones matmul for the
  sum-of-squares, ln/exp on ACT for rsqrt, and a broadcast matmul (with the
  norm weight folded in) to spread it back over partitions.
  Causality: tiles above the diagonal are skipped; boundary 128x128 blocks
  are masked by a triangular 0/1 multiply on GPSIMD after the exp.
All matmul inputs are float32r (TF32-like rounding, fp32 accumulation).
"""

import numpy as np

import concourse.bass as bass
import concourse.tile as tile
from concourse import bacc, mybir
from concourse.bass_utils import run_bass_kernel_spmd

F32 = mybir.dt.float32
F32R = mybir.dt.float32r
EXP = mybir.ActivationFunctionType.Exp
LN = mybir.ActivationFunctionType.Ln

EPS = 1e-6


def build_module(T=2048, with_qkbias=False, with_vbias=False, n_cores=8):
    """Build the per-core Bass module. D=1024, 8 heads of 64 dims per core."""
    D = 1024
    HG = 8          # heads per core
    HD = 64         # head dim
    DG = HG * HD    # 512 group dims
    NKT = T // 128  # k/t tiles
    NCH = T // 512  # q chunks
    SUB = 256       # xT streaming sub-chunk width

    nc = bacc.Bacc("TRN2", target_bir_lowering=False, debug=False,
                   num_devices=n_cores)

    xT_d = nc.dram_tensor("xt", [D, T], F32R, kind="ExternalInput")
    wq_d = nc.dram_tensor("wq", [D, DG], F32R, kind="ExternalInput")
    wk_d = nc.dram_tensor("wk", [D, DG], F32R, kind="ExternalInput")
    wv_d = nc.dram_tensor("wv", [D, DG], F32R, kind="ExternalInput")
    wo_d = nc.dram_tensor("wo", [DG, D], F32R, kind="ExternalInput")
    tri_d = nc.dram_tensor("tri", [128, 128], F32R, kind="ExternalInput")
    blk_d = nc.dram_tensor("blk", [128, 3, 66], F32R, kind="ExternalInput")
    bcqk_d = nc.dram_tensor("bcqk", [66, 256], F32R, kind="ExternalInput")
    vones_d = nc.dram_tensor("vones", [128, HG], F32R, kind="ExternalInput")
    if with_qkbias:
        bq_d = nc.dram_tensor("bq", [4, 128], F32, kind="ExternalInput")
        bk_d = nc.dram_tensor("bk", [4, 128], F32, kind="ExternalInput")
    if with_vbias:
        bv_d = nc.dram_tensor("bv", [1, DG], F32R, kind="ExternalInput")
        ones1_d = nc.dram_tensor("ones1", [1, 128], F32R, kind="ExternalInput")
    y_d = nc.dram_tensor("y", [T, D], F32, kind="ExternalOutput")

    with nc.allow_low_precision(reason="float32r matmul inputs"), \
         tile.TileContext(nc) as tc:
        with (
            tc.tile_pool(name="res", bufs=1) as res,
            tc.tile_pool(name="ktp", bufs=1) as ktp,
            tc.tile_pool(name="vtp", bufs=1) as vtp,
            tc.tile_pool(name="st2", bufs=2) as st2,
            tc.tile_pool(name="st3", bufs=3) as st3,
            tc.tile_pool(name="st5", bufs=5) as st5,
            tc.tile_pool(name="qtp", bufs=2) as qtp,
            tc.tile_pool(name="psbig", bufs=2, space="PSUM") as psbig,
            tc.tile_pool(name="psmid", bufs=2, space="PSUM") as psmid,
            tc.tile_pool(name="pso", bufs=2, space="PSUM") as pso,
        ):
            # ---- resident loads ----
            # (xT chunk-0 and wq are hoisted first so the first projection
            # matmuls aren't stuck behind the full weight download)
            xT_ap = xT_d.ap().rearrange("(a p) t -> p a t", p=128)
            wq_sb = res.tile([128, 8, DG], F32R, tag="wq")
            wk_sb = res.tile([128, 8, DG], F32R, tag="wk")
            wv_sb = res.tile([128, 8, DG], F32R, tag="wv")
            wo_sb = res.tile([128, 4, D], F32R, tag="wo")
            wq_ap = wq_d.ap().rearrange("(a p) m -> p a m", p=128)
            xts0 = []
            for s in range(2):
                xt = st3.tile([128, 8, SUB], F32R, tag="xt", bufs=2,
                              name=f"xt0_{s}")
                nc.sync.dma_start(out=xt[:], in_=xT_ap[:, :, s * SUB:(s + 1) * SUB])
                xts0.append(xt)
                # interleave so the first q-projection unit is unblocked early
                nc.sync.dma_start(out=wq_sb[:, :, s * 128:(s + 1) * 128],
                                  in_=wq_ap[:, :, s * 128:(s + 1) * 128])
            nc.sync.dma_start(out=wq_sb[:, :, 256:512], in_=wq_ap[:, :, 256:512])
            nc.sync.dma_start(out=wk_sb[:], in_=wk_d.ap().rearrange("(a p) m -> p a m", p=128))
            nc.sync.dma_start(out=wv_sb[:], in_=wv_d.ap().rearrange("(a p) m -> p a m", p=128))
            tri_sb = res.tile([128, 128], F32R, tag="tri")
            nc.sync.dma_start(out=tri_sb[:], in_=tri_d.ap())
            blk_sb = res.tile([128, 3, 66], F32R, tag="blk")
            nc.sync.dma_start(out=blk_sb[:], in_=blk_d.ap())
            bcqk_sb = res.tile([66, 256], F32R, tag="bcqk")
            nc.sync.dma_start(out=bcqk_sb[:], in_=bcqk_d.ap())
            vones_sb = res.tile([128, HG], F32R, tag="vones")
            nc.sync.dma_start(out=vones_sb[:], in_=vones_d.ap())
            eps_sb = res.tile([66, 1], F32, tag="eps")
            nc.vector.memset(eps_sb[:], EPS)
            nc.sync.dma_start(out=wo_sb[:], in_=wo_d.ap().rearrange("(a p) m -> p a m", p=128))
            bq_sb = bk_sb = bv_sb = ones1_sb = None
            if with_qkbias:
                bq_sb = res.tile([128, 4], F32, tag="bq")
                nc.sync.dma_start(out=bq_sb[:], in_=bq_d.ap().rearrange("m p -> p m"))
                bk_sb = res.tile([128, 4], F32, tag="bk")
                nc.sync.dma_start(out=bk_sb[:], in_=bk_d.ap().rearrange("m p -> p m"))
            if with_vbias:
                bv_sb = res.tile([1, DG], F32R, tag="bv")
                nc.sync.dma_start(out=bv_sb[:], in_=bv_d.ap())
                ones1_sb = res.tile([1, 128], F32R, tag="ones1")
                nc.sync.dma_start(out=ones1_sb[:], in_=ones1_d.ap())

            # resident kT [dims, T] (4 tiles) and v [t, dims+ones] (NKT tiles)
            kt_sb = [ktp.tile([128, T], F32R, tag=f"kt{m}", name=f"kt{m}") for m in range(4)]
            v_sb = [vtp.tile([128, HG, HD + 1], F32R, tag=f"v{t}", name=f"v{t}")
                    for t in range(NKT)]

            pending_outproj = []
            for c in range(NCH):
                # ---------- projection phase for chunk c ----------
                if c == 0:
                    xts = xts0
                else:
                    xts = []
                    for s in range(2):
                        xt = st3.tile([128, 8, SUB], F32R, tag="xt", bufs=2)
                        c0 = c * 512 + s * SUB
                        nc.sync.dma_start(out=xt[:], in_=xT_ap[:, :, c0:c0 + SUB])
                        xts.append(xt)

                qt_c = [qtp.tile([128, 512], F32R, tag=f"qt{m}", name=f"qt{m}", bufs=1)
                        for m in range(4)]

                # projection pipeline stages, skewed so PE never waits on
                # the DVE/ACT legs of the rms-norm chain.
                praw, psq = {}, {}

                def proj_qk(u):
                    which, m = u
                    w_sb = wq_sb if which == "q" else wk_sb
                    b_sb = bq_sb if which == "q" else bk_sb
                    ps = psmid.tile([128, 512], F32, tag="mid")
                    for s in range(2):
                        for k in range(8):
                            nc.tensor.matmul(
                                ps[:, s * SUB:(s + 1) * SUB],
                                w_sb[:, k, m * 128:(m + 1) * 128],
                                xts[s][:, k, :],
                                start=(k == 0), stop=(k == 7),
                            )
                    raw = st5.tile([128, 512], F32, tag="praw", bufs=8)
                    if b_sb is not None:
                        nc.vector.tensor_scalar_add(raw[:], ps[:],
                                                    b_sb[:, m:m + 1])
                    else:
                        nc.vector.tensor_copy(raw[:], ps[:])
                    sq = st3.tile([128, 512], F32R, tag="sq", bufs=2)
                    nc.vector.tensor_mul(sq[:], raw[:], raw[:])
                    praw[u] = raw
                    psq[u] = sq

                # rsqrt staging: units packed 3-per-tile at 32-aligned
                # partition bases (matmul bases must be 0/32/64). The ln/exp
                # run over the whole [66, 512] tile; rows between the packed
                # pairs are junk that is never read.
                rs_tiles = [st2.tile([66, 512], F32R, tag=f"rs{j}",
                                     name=f"rs{j}", bufs=1) for j in range(3)]
                ssq3 = [None, None, None]

                def rs_slice(i):
                    return rs_tiles[i // 3][32 * (i % 3):32 * (i % 3) + 2, :]

                def sumsq(i, u):
                    g, j = divmod(i, 3)
                    if j == 0:
                        ssq3[g] = pso.tile([66, 512], F32, tag="o",
                                           name=f"ssq3_{g}")
                    last = i in (2, 5, 7)
                    # blk3[:, j] spreads unit j's sums to rows 32j:32j+2 and
                    # zeros elsewhere, so the accumulated tile is fully
                    # written before the ln reads it.
                    nc.tensor.matmul(ssq3[g][:], blk_sb[:, j, :],
                                     psq[u][:], start=(j == 0), stop=last)
                    if last:
                        nc.scalar.activation(out=rs_tiles[g][:],
                                             in_=ssq3[g][:], func=LN,
                                             bias=eps_sb[:], scale=1.0 / HD)

                def rsqrt_all():
                    for g in range(3):
                        nc.scalar.activation(out=rs_tiles[g][:],
                                             in_=rs_tiles[g][:],
                                             func=EXP, scale=-0.5)

                def bcast_mul(i, u):
                    which, m = u
                    rsb = psbig.tile([128, 512], F32, tag="big")
                    b0 = 32 * (i % 3)
                    co = 0 if which == "q" else 128
                    nc.tensor.matmul(rsb[:],
                                     bcqk_sb[b0:b0 + 2, co:co + 128],
                                     rs_slice(i),
                                     start=True, stop=True)
                    if which == "q":
                        nc.vector.tensor_mul(qt_c[m][:], praw[u][:], rsb[:])
                    else:
                        nc.vector.tensor_mul(
                            kt_sb[m][:, c * 512:(c + 1) * 512],
                            praw[u][:], rsb[:])

                def proj_v(tt):
                    s, toff = divmod(tt * 128, SUB)
                    ps = psmid.tile([128, 512], F32, tag="mid")
                    for k in range(8):
                        nc.tensor.matmul(
                            ps[:],
                            xts[s][:, k, toff:toff + 128],
                            wv_sb[:, k, :],
                            start=(k == 0), stop=(k == 7 and not with_vbias),
                        )
                    if with_vbias:
                        nc.tensor.matmul(ps[:], ones1_sb[:], bv_sb[:],
                                         start=False, stop=True)
                    vt = v_sb[c * 4 + tt]
                    nc.vector.tensor_copy(
                        vt[:, :, 0:HD],
                        ps[:].rearrange("p (h d) -> p h d", h=HG),
                    )
                    nc.sync.dma_start(
                        out=vt[:, :, HD:HD + 1],
                        in_=vones_sb[:].rearrange("p (h o) -> p h o", o=1),
                    )

                units = [("q", m) for m in range(4)] + [("k", m) for m in range(4)]
                # proj(u_i) skewed with sumsq(u_{i-1}); the previous chunk's
                # deferred out-projection interleaves here (queues are quiet);
                # then v tiles (PE work covering the ACT ln/exp latency);
                # then the 8 bcast+muls.
                for i, u in enumerate(units):
                    proj_qk(u)
                    if i >= 1:
                        sumsq(i - 1, units[i - 1])
                    if pending_outproj:
                        pending_outproj.pop(0)()
                sumsq(len(units) - 1, units[-1])
                for tt in range(4):
                    proj_v(tt)
                    if pending_outproj:
                        pending_outproj.pop(0)()
                rsqrt_all()
                for i, u in enumerate(units):
                    bcast_mul(i, u)
                while pending_outproj:
                    pending_outproj.pop(0)()

                # ---------- attention phase for chunk c ----------
                ot_c = [qtp.tile([128, 512], F32R, tag=f"ot{m}", name=f"ot{m}", bufs=1)
                        for m in range(4)]
                for h in range(HG):
                    mt = h // 2
                    r0 = (h % 2) * 64
                    o_ps = pso.tile([HD + 1, 512], F32, tag="o")
                    n_full = 4 * c

                    def s_mm(sp, col0, kt, q0, width):
                        # S^T tile: [k 0:128, q q0:q0+width], K = head_dim 64
                        nc.tensor.matmul(
                            sp[:, col0:col0 + width],
                            kt_sb[mt][r0:r0 + 64, kt * 128:(kt + 1) * 128],
                            qt_c[mt][r0:r0 + 64, q0:q0 + width],
                            start=True, stop=True,
                        )

                    def av_mm(es, col0, kt, q0, width, first, last):
                        nc.tensor.matmul(
                            o_ps[:, q0:q0 + width],
                            v_sb[kt][:, h, :],
                            es[:, col0:col0 + width],
                            start=first, stop=last,
                        )

                    # S tiles: pairs of full k-tiles, then 4 diagonal tiles.
                    # AV of group g-1 is emitted after S/exp of group g so the
                    # PE stream stays ahead of ACT.
                    pend = None
                    pend_es = None
                    for p0 in range(0, n_full, 2):
                        sp = psbig.tile([128, 1024], F32, tag="big")
                        s_mm(sp, 0, p0, 0, 512)
                        s_mm(sp, 512, p0 + 1, 0, 512)
                        es = st3.tile([128, 1024], F32R, tag="es", bufs=2)
                        nc.scalar.activation(out=es[:], in_=sp[:], func=EXP,
                                             scale=0.125)
                        if pend is not None:
                            for (col0, kt_, q0_, w_, fi, la) in pend:
                                av_mm(pend_es, col0, kt_, q0_, w_, fi, la)
                        pend_es = es
                        pend = [(0, p0, 0, 512, p0 == 0, False),
                                (512, p0 + 1, 0, 512, False, False)]
                    # diagonal 512x512 block: the 4 boundary k-tiles are
                    # packed two-per-psum so the exp and mask ops batch:
                    #   group A: jj=0 (w 512) at cols 0:512, jj=3 (w 128) at
                    #            cols 512:640
                    #   group B: jj=1 (w 384) at cols 0:384, jj=2 (w 256) at
                    #            cols 512:768 (cols 384:512 junk, never read)
                    # Both groups have their triangular boundary blocks at
                    # cols 0:128 and 512:640 -> one strided mask multiply.
                    diag_groups = [
                        ((0, n_full + 0, 0, 512), (512, n_full + 3, 384, 128), 640),
                        ((0, n_full + 1, 128, 384), (512, n_full + 2, 256, 256), 768),
                    ]
                    for gi, (ga, gb, etot) in enumerate(diag_groups):
                        sp = psbig.tile([128, 1024], F32, tag="big")
                        for (col0, kt, q0, width) in (ga, gb):
                            s_mm(sp, col0, kt, q0, width)
                        es = st3.tile([128, 1024], F32R, tag="es", bufs=2)
                        if gi == 0:
                            nc.scalar.activation(out=es[:, 0:etot],
                                                 in_=sp[:, 0:etot],
                                                 func=EXP, scale=0.125)
                        else:
                            nc.scalar.activation(out=es[:, 0:384],
                                                 in_=sp[:, 0:384],
                                                 func=EXP, scale=0.125)
                            nc.scalar.activation(out=es[:, 512:768],
                                                 in_=sp[:, 512:768],
                                                 func=EXP, scale=0.125)
                        nc.vector.tensor_mul(
                            es[:].rearrange("p (a w) -> p a w", w=128)[:, 0:8:4, :],
                            es[:].rearrange("p (a w) -> p a w", w=128)[:, 0:8:4, :],
                            tri_sb[:].rearrange("p (o w) -> p o w", o=1).to_broadcast((128, 2, 128)),
                        )
                        if pend is not None:
                            for (col0, kt_, q0_, w_, fi, la) in pend:
                                av_mm(pend_es, col0, kt_, q0_, w_, fi, la)
                        pend_es = es
                        fi0 = (gi == 0 and n_full == 0)
                        pend = [(ga[0], ga[1], ga[2], ga[3], fi0, False),
                                (gb[0], gb[1], gb[2], gb[3], False, gi == 1)]
                    for (col0, kt_, q0_, w_, fi, la) in pend:
                        av_mm(pend_es, col0, kt_, q0_, w_, fi, la)

                    # softmax normalization + transposed attention output
                    recip = st2.tile([1, 512], F32, tag="recip", bufs=1)
                    nc.vector.reciprocal(out=recip[:], in_=o_ps[64:65, :])
                    recipb = st2.tile([64, 512], F32, tag="recipb", bufs=1)
                    nc.gpsimd.partition_broadcast(recipb[:], recip[:])
                    nc.vector.tensor_mul(ot_c[mt][r0:r0 + 64, :],
                                         o_ps[0:64, :], recipb[:])

                # ---------- out-projection for chunk c (deferred) ----------
                def make_outproj(cc, ots):
                    def one(tt, od):
                        def emit():
                            yp = psmid.tile([128, 512], F32, tag="mid",
                                            name=f"yp{cc}_{tt}_{od}")
                            for m in range(4):
                                nc.tensor.matmul(
                                    yp[:],
                                    ots[m][:, tt * 128:(tt + 1) * 128],
                                    wo_sb[:, m, od * 512:(od + 1) * 512],
                                    start=(m == 0), stop=(m == 3),
                                )
                            ysb = st2.tile([128, 512], F32, tag="y",
                                           name=f"ysb{cc}_{tt}_{od}")
                            nc.vector.tensor_copy(ysb[:], yp[:])
                            t0 = cc * 512 + tt * 128
                            nc.sync.dma_start(
                                out=y_d.ap()[t0:t0 + 128,
                                             od * 512:(od + 1) * 512],
                                in_=ysb[:])
                        return emit
                    return [one(tt, od) for tt in range(4) for od in range(2)]

                pending_outproj.extend(make_outproj(c, ot_c))
            while pending_outproj:
                pending_outproj.pop(0)()

    nc.compile()
    return nc


_CACHE = {}


def _get_module(T, with_qkbias, with_vbias, n_cores):
    key = (T, with_qkbias, with_vbias, n_cores)
    if key not in _CACHE:
        _CACHE[key] = build_module(T, with_qkbias, with_vbias, n_cores)
    return _CACHE[key]


def make_consts(qn_w, kn_w):
    HG = 8
    tri = np.triu(np.ones((128, 128), np.float32))   # keep k<=q: [i <= j]
    # blk[p, j, r] = 1 where r == 32j + p//64: unit-j sum-of-squares
    # selector covering all 66 output rows (zeros elsewhere).
    blk = np.zeros((128, 3, 66), np.float32)
    for j in range(3):
        blk[0:64, j, 32 * j] = 1.0
        blk[64:128, j, 32 * j + 1] = 1.0
    # broadcast lhsT replicated at partition bases 0/32/64 (PE needs
    # lhsT and rhs at the same base); cols 0:128 = qn, 128:256 = kn.
    bcqk = np.zeros((66, 256), np.float32)
    for j in range(3):
        for half in range(2):
            bcqk[32 * j + half, half * 64:(half + 1) * 64] = qn_w
            bcqk[32 * j + half, 128 + half * 64:128 + (half + 1) * 64] = kn_w
    vones = np.ones((128, HG), np.float32)
    return tri, blk, bcqk, vones


def make_in_maps(x, Wq, bq, Wk, bk, Wv, bv, Wo, qn_w, kn_w,
                 with_qkbias, with_vbias, n_cores=8):
    DG = 512
    tri, blk, bcqk, vones = make_consts(qn_w.astype(np.float32),
                                        kn_w.astype(np.float32))
    in_maps = []
    for c in range(n_cores):
        b, hg = divmod(c, 2)
        sl = slice(hg * DG, (hg + 1) * DG)
        im = {
            "xt": np.ascontiguousarray(x[b].T.astype(np.float32)),
            "wq": np.ascontiguousarray(Wq[sl, :].T.astype(np.float32)),
            "wk": np.ascontiguousarray(Wk[sl, :].T.astype(np.float32)),
            "wv": np.ascontiguousarray(Wv[sl, :].T.astype(np.float32)),
            "wo": np.ascontiguousarray(Wo[:, sl].T.astype(np.float32)),
            "tri": tri, "blk": blk, "bcqk": bcqk, "vones": vones,
        }
        if with_qkbias:
            im["bq"] = bq[sl].astype(np.float32).reshape(4, 128)
            im["bk"] = bk[sl].astype(np.float32).reshape(4, 128)
        if with_vbias:
            im["bv"] = bv[sl].astype(np.float32).reshape(1, DG)
            im["ones1"] = np.ones((1, 128), np.float32)
        in_maps.append(im)
    return in_maps


def kernel(x, Wq, bq, Wk, bk, Wv, bv, Wo, bo, qn_w, kn_w):
    x = np.asarray(x); Wq = np.asarray(Wq); Wk = np.asarray(Wk)
    Wv = np.asarray(Wv); Wo = np.asarray(Wo)
    bq = np.asarray(bq); bk = np.asarray(bk); bv = np.asarray(bv)
    bo = np.asarray(bo)
    qn_w = np.asarray(qn_w); kn_w = np.asarray(kn_w)
    B, T, D = x.shape

    with_qkbias = bool(np.any(bq != 0) or np.any(bk != 0))
    with_vbias = bool(np.any(bv != 0))
    nc = _get_module(T, with_qkbias, with_vbias, 8)
    in_maps = make_in_maps(x, Wq, bq, Wk, bk, Wv, bv, Wo, qn_w, kn_w,
                           with_qkbias, with_vbias, 8)
    res = run_bass_kernel_spmd(nc, in_maps, core_ids=list(range(8)))
    out = np.empty((B, T, D), np.float32)
    for b in range(B):
        out[b] = res.results[2 * b]["y"] + res.results[2 * b + 1]["y"]
    out += bo.astype(np.float32)
    return out


# revision 25
# speedup vs baseline: 1.3329x; 1.0216x over previous
"""Causal temporal attention (B=4, T=2048, D=1024, H=16, hd=64) on 8 trn2 cores.

Sharding: core c handles batch b=c//2 and head-group hg=c%2 (8 heads, 512 dims).
Each core computes y_partial[b] = attn_out_g @ Wo_g.T for its head group; the
host sums the two partials per batch and adds bo.

Per-core dataflow:
  xT [1024, 2048] (host-pretransposed x[b]) streams in 256-col sub-chunks.
  qT,kT are computed transposed [512, T] (dims on partitions) so the S matmul
  contracts head_dim on partitions; v is computed natural [T, 512] with an
  appended ones-column per head so the AV matmul also produces the softmax
  denominator (row 64 of the [65, 512] accumulator).
  RMS-norm over head_dim (= partitions) uses a block-ones matmul for the
  sum-of-squares, ln/exp on ACT for rsqrt, and a broadcast matmul (with the
  norm weight folded in) to spread it back over partitions.
  Causality: tiles above the diagonal are skipped; boundary 128x128 blocks
  are masked by a triangular 0/1 multiply on GPSIMD after the exp.
All matmul inputs are float32r (TF32-like rounding, fp32 accumulation).
"""

import numpy as np

import concourse.bass as bass
import concourse.tile as tile
from concourse import bacc, mybir
from concourse.bass_utils import run_bass_kernel_spmd
from concourse import bass2jax

F32 = mybir.dt.float32
F32R = mybir.dt.float32r
EXP = mybir.ActivationFunctionType.Exp
LN = mybir.ActivationFunctionType.Ln

EPS = 1e-6


def build_module(T=2048, with_qkbias=False, with_vbias=False, n_cores=8):
    """Build the per-core Bass module. D=1024, 8 heads of 64 dims per core."""
    D = 1024
    HG = 8          # heads per core
    HD = 64         # head dim
    DG = HG * HD    # 512 group dims
    NKT = T // 128  # k/t tiles
    NCH = T // 512  # q chunks
    SUB = 256       # xT streaming sub-chunk width

    nc = bacc.Bacc("TRN2", target_bir_lowering=False, debug=False,
                   num_devices=n_cores)

    xT_d = nc.dram_tensor("xt", [D, T], F32R, kind="ExternalInput")
    wq_d = nc.dram_tensor("wq", [D, DG], F32R, kind="ExternalInput")
    wk_d = nc.dram_tensor("wk", [D, DG], F32R, kind="ExternalInput")
    wv_d = nc.dram_tensor("wv", [D, DG], F32R, kind="ExternalInput")
    wo_d = nc.dram_tensor("wo", [DG, D], F32R, kind="ExternalInput")
    tri_d = nc.dram_tensor("tri", [128, 128], F32R, kind="ExternalInput")
    blk_d = nc.dram_tensor("blk", [128, 3, 66], F32R, kind="ExternalInput")
    bcqk_d = nc.dram_tensor("bcqk", [66, 256], F32R, kind="ExternalInput")
    vones_d = nc.dram_tensor("vones", [128, HG], F32R, kind="ExternalInput")
    if with_qkbias:
        bq_d = nc.dram_tensor("bq", [4, 128], F32, kind="ExternalInput")
        bk_d = nc.dram_tensor("bk", [4, 128], F32, kind="ExternalInput")
    if with_vbias:
        bv_d = nc.dram_tensor("bv", [1, DG], F32R, kind="ExternalInput")
        ones1_d = nc.dram_tensor("ones1", [1, 128], F32R, kind="ExternalInput")
    y_d = nc.dram_tensor("y", [T, D], F32, kind="ExternalOutput")

    with nc.allow_low_precision(reason="float32r matmul inputs"), \
         tile.TileContext(nc) as tc:
        with (
            tc.tile_pool(name="res", bufs=1) as res,
            tc.tile_pool(name="ktp", bufs=1) as ktp,
            tc.tile_pool(name="vtp", bufs=1) as vtp,
            tc.tile_pool(name="st2", bufs=2) as st2,
            tc.tile_pool(name="st3", bufs=3) as st3,
            tc.tile_pool(name="st5", bufs=5) as st5,
            tc.tile_pool(name="qtp", bufs=2) as qtp,
            tc.tile_pool(name="psbig", bufs=2, space="PSUM") as psbig,
            tc.tile_pool(name="psmid", bufs=2, space="PSUM") as psmid,
            tc.tile_pool(name="pso", bufs=2, space="PSUM") as pso,
        ):
            # ---- resident loads ----
            # (xT chunk-0 and wq are hoisted first so the first projection
            # matmuls aren't stuck behind the full weight download)
            xT_ap = xT_d.ap().rearrange("(a p) t -> p a t", p=128)
            wq_sb = res.tile([128, 8, DG], F32R, tag="wq")
            wk_sb = res.tile([128, 8, DG], F32R, tag="wk")
            wv_sb = res.tile([128, 8, DG], F32R, tag="wv")
            wo_sb = res.tile([128, 4, D], F32R, tag="wo")
            wq_ap = wq_d.ap().rearrange("(a p) m -> p a m", p=128)
            xts0 = []
            for s in range(2):
                xt = st3.tile([128, 8, SUB], F32R, tag="xt", bufs=2,
                              name=f"xt0_{s}")
                nc.sync.dma_start(out=xt[:], in_=xT_ap[:, :, s * SUB:(s + 1) * SUB])
                xts0.append(xt)
                # interleave so the first q-projection unit is unblocked early
                nc.sync.dma_start(out=wq_sb[:, :, s * 128:(s + 1) * 128],
                                  in_=wq_ap[:, :, s * 128:(s + 1) * 128])
            nc.sync.dma_start(out=wq_sb[:, :, 256:512], in_=wq_ap[:, :, 256:512])
            nc.sync.dma_start(out=wk_sb[:], in_=wk_d.ap().rearrange("(a p) m -> p a m", p=128))
            nc.sync.dma_start(out=wv_sb[:], in_=wv_d.ap().rearrange("(a p) m -> p a m", p=128))
            tri_sb = res.tile([128, 128], F32R, tag="tri")
            nc.sync.dma_start(out=tri_sb[:], in_=tri_d.ap())
            blk_sb = res.tile([128, 3, 66], F32R, tag="blk")
            nc.sync.dma_start(out=blk_sb[:], in_=blk_d.ap())
            bcqk_sb = res.tile([66, 256], F32R, tag="bcqk")
            nc.sync.dma_start(out=bcqk_sb[:], in_=bcqk_d.ap())
            vones_sb = res.tile([128, HG], F32R, tag="vones")
            nc.sync.dma_start(out=vones_sb[:], in_=vones_d.ap())
            eps_sb = res.tile([66, 1], F32, tag="eps")
            nc.vector.memset(eps_sb[:], EPS)
            nc.sync.dma_start(out=wo_sb[:], in_=wo_d.ap().rearrange("(a p) m -> p a m", p=128))
            bq_sb = bk_sb = bv_sb = ones1_sb = None
            if with_qkbias:
                bq_sb = res.tile([128, 4], F32, tag="bq")
                nc.sync.dma_start(out=bq_sb[:], in_=bq_d.ap().rearrange("m p -> p m"))
                bk_sb = res.tile([128, 4], F32, tag="bk")
                nc.sync.dma_start(out=bk_sb[:], in_=bk_d.ap().rearrange("m p -> p m"))
            if with_vbias:
                bv_sb = res.tile([1, DG], F32R, tag="bv")
                nc.sync.dma_start(out=bv_sb[:], in_=bv_d.ap())
                ones1_sb = res.tile([1, 128], F32R, tag="ones1")
                nc.sync.dma_start(out=ones1_sb[:], in_=ones1_d.ap())

            # resident kT [dims, T] (4 tiles) and v [t, dims+ones] (NKT tiles)
            kt_sb = [ktp.tile([128, T], F32R, tag=f"kt{m}", name=f"kt{m}") for m in range(4)]
            v_sb = [vtp.tile([128, HG, HD + 1], F32R, tag=f"v{t}", name=f"v{t}")
                    for t in range(NKT)]

            pending_outproj = []
            for c in range(NCH):
                # ---------- projection phase for chunk c ----------
                if c == 0:
                    xts = xts0
                else:
                    xts = []
                    for s in range(2):
                        xt = st3.tile([128, 8, SUB], F32R, tag="xt", bufs=2)
                        c0 = c * 512 + s * SUB
                        nc.sync.dma_start(out=xt[:], in_=xT_ap[:, :, c0:c0 + SUB])
                        xts.append(xt)

                qt_c = [qtp.tile([128, 512], F32R, tag=f"qt{m}", name=f"qt{m}", bufs=1)
                        for m in range(4)]

                # projection pipeline stages, skewed so PE never waits on
                # the DVE/ACT legs of the rms-norm chain.
                praw, psq = {}, {}

                def proj_qk(u):
                    which, m = u
                    w_sb = wq_sb if which == "q" else wk_sb
                    b_sb = bq_sb if which == "q" else bk_sb
                    ps = psmid.tile([128, 512], F32, tag="mid")
                    for s in range(2):
                        for k in range(8):
                            nc.tensor.matmul(
                                ps[:, s * SUB:(s + 1) * SUB],
                                w_sb[:, k, m * 128:(m + 1) * 128],
                                xts[s][:, k, :],
                                start=(k == 0), stop=(k == 7),
                            )
                    raw = st5.tile([128, 512], F32, tag="praw", bufs=8)
                    if b_sb is not None:
                        nc.vector.tensor_scalar_add(raw[:], ps[:],
                                                    b_sb[:, m:m + 1])
                    else:
                        nc.vector.tensor_copy(raw[:], ps[:])
                    sq = st3.tile([128, 512], F32R, tag="sq", bufs=2)
                    nc.vector.tensor_mul(sq[:], raw[:], raw[:])
                    praw[u] = raw
                    psq[u] = sq

                # rsqrt staging: units packed 3-per-tile at 32-aligned
                # partition bases (matmul bases must be 0/32/64). The ln/exp
                # run over the whole [66, 512] tile; rows between the packed
                # pairs are junk that is never read.
                rs_tiles = [st2.tile([66, 512], F32R, tag=f"rs{j}",
                                     name=f"rs{j}", bufs=1) for j in range(3)]
                ssq3 = [None, None, None]

                def rs_slice(i):
                    return rs_tiles[i // 3][32 * (i % 3):32 * (i % 3) + 2, :]

                def sumsq(i, u):
                    g, j = divmod(i, 3)
                    if j == 0:
                        ssq3[g] = pso.tile([66, 512], F32, tag="o",
                                           name=f"ssq3_{g}")
                    last = i in (2, 5, 7)
                    # blk3[:, j] spreads unit j's sums to rows 32j:32j+2 and
                    # zeros elsewhere, so the accumulated tile is fully
                    # written before the ln reads it.
                    nc.tensor.matmul(ssq3[g][:], blk_sb[:, j, :],
                                     psq[u][:], start=(j == 0), stop=last)
                    if last:
                        nc.scalar.activation(out=rs_tiles[g][:],
                                             in_=ssq3[g][:], func=LN,
                                             bias=eps_sb[:], scale=1.0 / HD)

                def rsqrt_all():
                    for g in range(3):
                        nc.scalar.activation(out=rs_tiles[g][:],
                                             in_=rs_tiles[g][:],
                                             func=EXP, scale=-0.5)

                def bcast_mul(i, u):
                    which, m = u
                    rsb = psbig.tile([128, 512], F32, tag="big")
                    b0 = 32 * (i % 3)
                    co = 0 if which == "q" else 128
                    nc.tensor.matmul(rsb[:],
                                     bcqk_sb[b0:b0 + 2, co:co + 128],
                                     rs_slice(i),
                                     start=True, stop=True)
                    if which == "q":
                        nc.vector.tensor_mul(qt_c[m][:], praw[u][:], rsb[:])
                    else:
                        nc.vector.tensor_mul(
                            kt_sb[m][:, c * 512:(c + 1) * 512],
                            praw[u][:], rsb[:])

                def proj_v(tt):
                    s, toff = divmod(tt * 128, SUB)
                    ps = psmid.tile([128, 512], F32, tag="mid")
                    for k in range(8):
                        nc.tensor.matmul(
                            ps[:],
                            xts[s][:, k, toff:toff + 128],
                            wv_sb[:, k, :],
                            start=(k == 0), stop=(k == 7 and not with_vbias),
                        )
                    if with_vbias:
                        nc.tensor.matmul(ps[:], ones1_sb[:], bv_sb[:],
                                         start=False, stop=True)
                    vt = v_sb[c * 4 + tt]
                    nc.vector.tensor_copy(
                        vt[:, :, 0:HD],
                        ps[:].rearrange("p (h d) -> p h d", h=HG),
                    )
                    nc.sync.dma_start(
                        out=vt[:, :, HD:HD + 1],
                        in_=vones_sb[:].rearrange("p (h o) -> p h o", o=1),
                    )

                units = [("q", m) for m in range(4)] + [("k", m) for m in range(4)]
                # proj(u_i) skewed with sumsq(u_{i-1}); the previous chunk's
                # deferred out-projection interleaves here (queues are quiet);
                # then v tiles (PE work covering the ACT ln/exp latency);
                # then the 8 bcast+muls.
                for i, u in enumerate(units):
                    proj_qk(u)
                    if i >= 1:
                        sumsq(i - 1, units[i - 1])
                    if pending_outproj:
                        pending_outproj.pop(0)()
                sumsq(len(units) - 1, units[-1])
                for tt in range(4):
                    proj_v(tt)
                    if pending_outproj:
                        pending_outproj.pop(0)()
                rsqrt_all()
                for i, u in enumerate(units):
                    bcast_mul(i, u)
                while pending_outproj:
                    pending_outproj.pop(0)()

                # ---------- attention phase for chunk c ----------
                ot_c = [qtp.tile([128, 512], F32R, tag=f"ot{m}", name=f"ot{m}", bufs=1)
                        for m in range(4)]
                for h in range(HG):
                    mt = h // 2
                    r0 = (h % 2) * 64
                    o_ps = pso.tile([HD + 1, 512], F32, tag="o")
                    n_full = 4 * c

                    def s_mm(sp, col0, kt, q0, width):
                        # S^T tile: [k 0:128, q q0:q0+width], K = head_dim 64
                        nc.tensor.matmul(
                            sp[:, col0:col0 + width],
                            kt_sb[mt][r0:r0 + 64, kt * 128:(kt + 1) * 128],
                            qt_c[mt][r0:r0 + 64, q0:q0 + width],
                            start=True, stop=True,
                        )

                    def av_mm(es, col0, kt, q0, width, first, last):
                        nc.tensor.matmul(
                            o_ps[:, q0:q0 + width],
                            v_sb[kt][:, h, :],
                            es[:, col0:col0 + width],
                            start=first, stop=last,
                        )

                    # S tiles: pairs of full k-tiles, then 4 diagonal tiles.
                    # AV of group g-1 is emitted after S/exp of group g so the
                    # PE stream stays ahead of ACT.
                    pend = None
                    pend_es = None
                    for p0 in range(0, n_full, 2):
                        sp = psbig.tile([128, 1024], F32, tag="big")
                        s_mm(sp, 0, p0, 0, 512)
                        s_mm(sp, 512, p0 + 1, 0, 512)
                        es = st3.tile([128, 1024], F32R, tag="es", bufs=2)
                        nc.scalar.activation(out=es[:], in_=sp[:], func=EXP,
                                             scale=0.125)
                        if pend is not None:
                            for (col0, kt_, q0_, w_, fi, la) in pend:
                                av_mm(pend_es, col0, kt_, q0_, w_, fi, la)
                        pend_es = es
                        pend = [(0, p0, 0, 512, p0 == 0, False),
                                (512, p0 + 1, 0, 512, False, False)]
                    # diagonal 512x512 block: the 4 boundary k-tiles are
                    # packed two-per-psum, contiguously, so one exp covers
                    # each pair:
                    #   group A: jj=0 (w 512) at cols 0:512, jj=2 (w 256) at
                    #            cols 512:768 -> exp over 0:768
                    #   group B: jj=1 (w 384) at cols 0:384, jj=3 (w 128) at
                    #            cols 384:512 -> exp over 0:512
                    # Triangular boundary blocks sit at 128-col blocks
                    # {0, 4} for A and {0, 3} for B -> one strided mask
                    # multiply per group.
                    diag_groups = [
                        ((0, n_full + 0, 0, 512), (512, n_full + 2, 256, 256),
                         768, 4),
                        ((0, n_full + 1, 128, 384), (384, n_full + 3, 384, 128),
                         512, 3),
                    ]
                    for gi, (ga, gb, etot, bstep) in enumerate(diag_groups):
                        sp = psbig.tile([128, 1024], F32, tag="big")
                        for (col0, kt, q0, width) in (ga, gb):
                            s_mm(sp, col0, kt, q0, width)
                        es = st3.tile([128, 1024], F32R, tag="es", bufs=2)
                        nc.scalar.activation(out=es[:, 0:etot],
                                             in_=sp[:, 0:etot],
                                             func=EXP, scale=0.125)
                        esb = es[:].rearrange("p (a w) -> p a w", w=128)
                        nc.vector.tensor_mul(
                            esb[:, 0:bstep + 1:bstep, :],
                            esb[:, 0:bstep + 1:bstep, :],
                            tri_sb[:].rearrange("p (o w) -> p o w", o=1).to_broadcast((128, 2, 128)),
                        )
                        if pend is not None:
                            for (col0, kt_, q0_, w_, fi, la) in pend:
                                av_mm(pend_es, col0, kt_, q0_, w_, fi, la)
                        pend_es = es
                        fi0 = (gi == 0 and n_full == 0)
                        pend = [(ga[0], ga[1], ga[2], ga[3], fi0, False),
                                (gb[0], gb[1], gb[2], gb[3], False, gi == 1)]
                    for (col0, kt_, q0_, w_, fi, la) in pend:
                        av_mm(pend_es, col0, kt_, q0_, w_, fi, la)

                    # softmax normalization + transposed attention output
                    recip = st2.tile([1, 512], F32, tag="recip", bufs=1)
                    nc.vector.reciprocal(out=recip[:], in_=o_ps[64:65, :])
                    recipb = st2.tile([64, 512], F32, tag="recipb", bufs=1)
                    nc.gpsimd.partition_broadcast(recipb[:], recip[:])
                    nc.vector.tensor_mul(ot_c[mt][r0:r0 + 64, :],
                                         o_ps[0:64, :], recipb[:])

                # ---------- out-projection for chunk c (deferred) ----------
                def make_outproj(cc, ots):
                    def one(tt, od):
                        def emit():
                            yp = psmid.tile([128, 512], F32, tag="mid",
                                            name=f"yp{cc}_{tt}_{od}")
                            for m in range(4):
                                nc.tensor.matmul(
                                    yp[:],
                                    ots[m][:, tt * 128:(tt + 1) * 128],
                                    wo_sb[:, m, od * 512:(od + 1) * 512],
                                    start=(m == 0), stop=(m == 3),
                                )
                            ysb = st2.tile([128, 512], F32, tag="y",
                                           name=f"ysb{cc}_{tt}_{od}")
                            nc.vector.tensor_copy(ysb[:], yp[:])
                            t0 = cc * 512 + tt * 128
                            nc.sync.dma_start(
                                out=y_d.ap()[t0:t0 + 128,
                                             od * 512:(od + 1) * 512],
                                in_=ysb[:])
                        return emit
                    return [one(tt, od) for tt in range(4) for od in range(2)]

                pending_outproj.extend(make_outproj(c, ot_c))
            while pending_outproj:
                pending_outproj.pop(0)()

    nc.compile()
    return nc


_CACHE = {}


def _get_module(T, with_qkbias, with_vbias, n_cores):
    key = (T, with_qkbias, with_vbias, n_cores)
    if key not in _CACHE:
        _CACHE[key] = build_module(T, with_qkbias, with_vbias, n_cores)
    return _CACHE[key]


def make_consts(qn_w, kn_w):
    HG = 8
    tri = np.triu(np.ones((128, 128), np.float32))   # keep k<=q: [i <= j]
    # blk[p, j, r] = 1 where r == 32j + p//64: unit-j sum-of-squares
    # selector covering all 66 output rows (zeros elsewhere).
    blk = np.zeros((128, 3, 66), np.float32)
    for j in range(3):
        blk[0:64, j, 32 * j] = 1.0
        blk[64:128, j, 32 * j + 1] = 1.0
    # broadcast lhsT replicated at partition bases 0/32/64 (PE needs
    # lhsT and rhs at the same base); cols 0:128 = qn, 128:256 = kn.
    bcqk = np.zeros((66, 256), np.float32)
    for j in range(3):
        for half in range(2):
            bcqk[32 * j + half, half * 64:(half + 1) * 64] = qn_w
            bcqk[32 * j + half, 128 + half * 64:128 + (half + 1) * 64] = kn_w
    vones = np.ones((128, HG), np.float32)
    return tri, blk, bcqk, vones


def make_in_maps(x, Wq, bq, Wk, bk, Wv, bv, Wo, qn_w, kn_w,
                 with_qkbias, with_vbias, n_cores=8):
    DG = 512
    tri, blk, bcqk, vones = make_consts(qn_w.astype(np.float32),
                                        kn_w.astype(np.float32))
    in_maps = []
    for c in range(n_cores):
        b, hg = divmod(c, 2)
        sl = slice(hg * DG, (hg + 1) * DG)
        im = {
            "xt": np.ascontiguousarray(x[b].T.astype(np.float32)),
            "wq": np.ascontiguousarray(Wq[sl, :].T.astype(np.float32)),
            "wk": np.ascontiguousarray(Wk[sl, :].T.astype(np.float32)),
            "wv": np.ascontiguousarray(Wv[sl, :].T.astype(np.float32)),
            "wo": np.ascontiguousarray(Wo[:, sl].T.astype(np.float32)),
            "tri": tri, "blk": blk, "bcqk": bcqk, "vones": vones,
        }
        if with_qkbias:
            im["bq"] = bq[sl].astype(np.float32).reshape(4, 128)
            im["bk"] = bk[sl].astype(np.float32).reshape(4, 128)
        if with_vbias:
            im["bv"] = bv[sl].astype(np.float32).reshape(1, DG)
            im["ones1"] = np.ones((1, 128), np.float32)
        in_maps.append(im)
    return in_maps


_RUNNER_CACHE = {}


def _run_cached(nc, in_maps, key):
    """run_bass_via_pjrt with the jitted executable cached across calls."""
    import jax
    from jax.sharding import Mesh, PartitionSpec
    from jax.experimental.shard_map import shard_map
    from concourse import mybir as _mb

    n_cores = len(in_maps)
    if key not in _RUNNER_CACHE:
        bass2jax.install_neuronx_cc_hook()
        part_name = (nc.partition_id_tensor.name
                     if nc.partition_id_tensor else None)
        in_names, out_names, out_avals = [], [], []
        for alloc in nc.m.functions[0].allocations:
            if not isinstance(alloc, _mb.MemoryLocationSet):
                continue
            name = alloc.memorylocations[0].name
            if alloc.kind == "ExternalInput":
                if name != part_name:
                    in_names.append(name)
            elif alloc.kind == "ExternalOutput":
                out_names.append(name)
                out_avals.append(jax.core.ShapedArray(
                    tuple(alloc.tensor_shape), _mb.dt.np(alloc.dtype)))
        n_params = len(in_names)
        all_names = in_names + out_names
        if part_name is not None:
            all_names = all_names + [part_name]

        def _body(*args):
            operands = list(args)
            if part_name is not None:
                operands.append(bass2jax.partition_id_tensor())
            outs = bass2jax._bass_exec_p.bind(
                *operands, out_avals=tuple(out_avals),
                in_names=tuple(all_names), out_names=tuple(out_names),
                lowering_input_output_aliases=(),
                sim_require_finite=True, sim_require_nnan=True, nc=nc)
            return tuple(outs)

        devices = jax.devices()[:n_cores]
        mesh = Mesh(np.asarray(devices), ("core",))
        n_outs = len(out_names)
        sharded = jax.jit(
            shard_map(_body, mesh=mesh,
                      in_specs=(PartitionSpec("core"),) * (n_params + n_outs),
                      out_specs=(PartitionSpec("core"),) * n_outs,
                      check_rep=False),
            donate_argnums=tuple(range(n_params, n_params + n_outs)),
            keep_unused=True)
        _RUNNER_CACHE[key] = (sharded, in_names, out_names, out_avals)

    sharded, in_names, out_names, out_avals = _RUNNER_CACHE[key]
    concat_in = [np.concatenate([np.asarray(m[nm]) for m in in_maps], axis=0)
                 for nm in in_names]
    concat_zeros = [np.zeros((n_cores * a.shape[0], *a.shape[1:]), a.dtype)
                    for a in out_avals]
    out_arrs = sharded(*concat_in, *concat_zeros)
    return [
        {nm: np.asarray(out_arrs[i]).reshape(n_cores, *out_avals[i].shape)[c]
         for i, nm in enumerate(out_names)}
        for c in range(n_cores)
    ]


def kernel(x, Wq, bq, Wk, bk, Wv, bv, Wo, bo, qn_w, kn_w):
    x = np.asarray(x); Wq = np.asarray(Wq); Wk = np.asarray(Wk)
    Wv = np.asarray(Wv); Wo = np.asarray(Wo)
    bq = np.asarray(bq); bk = np.asarray(bk); bv = np.asarray(bv)
    bo = np.asarray(bo)
    qn_w = np.asarray(qn_w); kn_w = np.asarray(kn_w)
    B, T, D = x.shape

    with_qkbias = bool(np.any(bq != 0) or np.any(bk != 0))
    with_vbias = bool(np.any(bv != 0))
    nc = _get_module(T, with_qkbias, with_vbias, 8)
    in_maps = make_in_maps(x, Wq, bq, Wk, bk, Wv, bv, Wo, qn_w, kn_w,
                           with_qkbias, with_vbias, 8)
    key = (T, with_qkbias, with_vbias, 8)
    results = _run_cached(nc, in_maps, key)
    out = np.empty((B, T, D), np.float32)
    for b in range(B):
        out[b] = results[2 * b]["y"] + results[2 * b + 1]["y"]
    out += bo.astype(np.float32)
    return out
